# revision 1
# baseline (speedup 1.0000x reference)
"""GCN conv (linear -> weighted gather -> segment-sum by dst) on 8 trn2 cores.

Math: out = segment_sum((x @ W.T + b)[src] * w[:, None], dst, N)

Strategy per core (nodes range-partitioned, edges partitioned by dst):
  - Host sorts each core's edges by dst and groups them into 128-node dst
    blocks; each block's edge list is padded to B_MAX chunks of 128 edges.
  - Device gathers x rows by src (indirect DMA, one 128-row chunk per
    call), appends a ones column, and uses a weighted one-hot matmul to
    segment-sum pre-linear features:
        ST[feat, node] += sum_p gx[p, feat] * (w_p * [dst_p == node])
    giving S_ext = [segsum(w*x) | segsum(w)] per block, pre-transposed.
  - A second matmul applies the linear layer: out = S_ext @ [W | b]^T.
"""

import bass_rust
import numpy as np

from concourse import bass, mybir, tile
from concourse.bass_utils import run_bass_kernel_spmd

P = 128
NCORES = 8
N, E, D = 100000, 1200000, 64
NODES_PER_CORE = N // NCORES  # 12500
NB = (NODES_PER_CORE + P - 1) // P  # 98 blocks of 128 dst nodes
NPAD = NB * P  # 12544

_wait_counter = [0]


def _split_multi_waits(nc):
    """Installed walrus rejects >1 sync wait per instruction; park excess
    waits on fresh single-wait NoOps inserted before the owner (same
    engine, so in-order execution preserves semantics)."""
    for fn in nc.m.functions:
        for bb in fn.blocks:
            insts = bb.instructions
            if not any(
                i.sync_info is not None and len(i.sync_info.on_wait) > 1
                for i in insts
            ):
                continue
            out = []
            for inst in insts:
                si = inst.sync_info
                waits = list(si.on_wait) if si is not None else []
                if len(waits) > 1:
                    for wv in waits[:-1]:
                        _wait_counter[0] += 1
                        nop = mybir.InstNoOp(
                            name=f"waitsplit-{_wait_counter[0]}",
                            engine=inst.engine,
                        )
                        nop.sync_info = bass_rust.SyncInfo(
                            on_wait=[wv], on_update=[]
                        )
                        out.append(nop)
                    inst.sync_info = bass_rust.SyncInfo(
                        on_wait=[waits[-1]], on_update=list(si.on_update)
                    )
                out.append(inst)
            bb.instructions = out


class _TC(tile.TileContext):
    def __exit__(self, *args):
        ret = super().__exit__(*args)
        _split_multi_waits(self.nc)
        return ret


def _build_program(B_max: int):
    C = NB * B_max  # chunks (of 128 edges) per core
    f32 = mybir.dt.float32
    nc = bass.Bass()
    x_p = nc.declare_dram_parameter("x", [N, D], f32, isOutput=False)
    srcT_p = nc.declare_dram_parameter("srcT", [P, C], mybir.dt.int32, isOutput=False)
    relT_p = nc.declare_dram_parameter("relT", [P, C], f32, isOutput=False)
    wT_p = nc.declare_dram_parameter("wT", [P, C], f32, isOutput=False)
    wext_p = nc.declare_dram_parameter("wext", [D + 1, D], f32, isOutput=False)
    iota_p = nc.declare_dram_parameter("iota", [P, P], f32, isOutput=False)
    out_p = nc.declare_dram_parameter("out", [NPAD, D], f32, isOutput=True)

    with _TC(nc) as tc:
        with (
            tc.tile_pool(name="const", bufs=1) as cpool,
            tc.tile_pool(name="gx", bufs=3) as gxpool,
            tc.tile_pool(name="oh", bufs=4) as ohpool,
            tc.tile_pool(name="stsb", bufs=2) as stpool,
            tc.tile_pool(name="outsb", bufs=3) as opool,
            tc.tile_pool(name="pst", bufs=2, space="PSUM") as pstpool,
            tc.tile_pool(name="pout", bufs=2, space="PSUM") as poutpool,
        ):
            iota_sb = cpool.tile([P, P], f32)
            nc.sync.dma_start(out=iota_sb[:], in_=iota_p[:])
            wext_sb = cpool.tile([D + 1, D], f32)
            nc.sync.dma_start(out=wext_sb[:], in_=wext_p[:])
            # one-shot [128, C] loads of this size crash neuronxcc's
            # DataLocalityOpt; slice them into <=98-column pieces
            srcT_sb = cpool.tile([P, C], mybir.dt.int32)
            relT_sb = cpool.tile([P, C], f32)
            wT_sb = cpool.tile([P, C], f32)
            for s in range(0, C, 98):
                e = min(C, s + 98)
                nc.sync.dma_start(out=srcT_sb[:, s:e], in_=srcT_p[:, s:e])
                nc.sync.dma_start(out=relT_sb[:, s:e], in_=relT_p[:, s:e])
                nc.sync.dma_start(out=wT_sb[:, s:e], in_=wT_p[:, s:e])

            for blk in range(NB):
                # gather this block's src rows: gx[p, j, :D] = x[srcT[p, blk*B_max+j]]
                gx = gxpool.tile([P, B_max, D + 1], f32)
                for j in range(B_max):
                    cc = blk * B_max + j
                    nc.gpsimd.indirect_dma_start(
                        out=gx[:, j, 0:D],
                        out_offset=None,
                        in_=x_p[:],
                        in_offset=bass.IndirectOffsetOnAxis(
                            ap=srcT_sb[:, cc : cc + 1],
                            axis=0,
                        ),
                    )
                nc.vector.memset(gx[:, :, D : D + 1], 1.0)

                pst = pstpool.tile([D + 1, P], f32)
                for j in range(B_max):
                    cc = blk * B_max + j
                    oh = ohpool.tile([P, P], f32)
                    # oh[p, f] = w[p] * (rel_dst[p] == f)
                    nc.vector.tensor_scalar(
                        out=oh[:],
                        in0=iota_sb[:],
                        scalar1=relT_sb[:, cc : cc + 1],
                        scalar2=wT_sb[:, cc : cc + 1],
                        op0=mybir.AluOpType.is_equal,
                        op1=mybir.AluOpType.mult,
                    )
                    # pst[feat, node] += sum_p gx[p, j, feat] * oh[p, node]
                    nc.tensor.matmul(
                        pst[:],
                        lhsT=gx[:, j, :],
                        rhs=oh[:],
                        start=(j == 0),
                        stop=(j == B_max - 1),
                    )
                st_sb = stpool.tile([D + 1, P], f32)
                nc.any.tensor_copy(out=st_sb[:], in_=pst[:])
                pout = poutpool.tile([P, D], f32)
                # out[node, dout] = sum_k st[k, node] * wext[k, dout]
                nc.tensor.matmul(
                    pout[:], lhsT=st_sb[:], rhs=wext_sb[:], start=True, stop=True
                )
                out_sb = opool.tile([P, D], f32)
                nc.any.tensor_copy(out=out_sb[:], in_=pout[:])
                nc.sync.dma_start(out=out_p[blk * P : (blk + 1) * P, :], in_=out_sb[:])
    return nc


def kernel(x, src, dst, w, W, b):
    x = np.ascontiguousarray(np.asarray(x, dtype=np.float32))
    src = np.asarray(src).astype(np.int32)
    dst = np.asarray(dst).astype(np.int32)
    w = np.asarray(w, dtype=np.float32)
    W = np.asarray(W, dtype=np.float32)
    b = np.asarray(b, dtype=np.float32)

    core_of = dst // NODES_PER_CORE
    per_core = []
    max_cnt = 1
    for c in range(NCORES):
        m = core_of == c
        s_c = src[m]
        d_c = (dst[m] - c * NODES_PER_CORE).astype(np.int32)
        w_c = w[m]
        order = np.argsort(d_c, kind="stable")
        s_c, d_c, w_c = s_c[order], d_c[order], w_c[order]
        blk = d_c >> 7
        counts = np.bincount(blk, minlength=NB).astype(np.int64)
        per_core.append((s_c, d_c, w_c, blk, counts))
        if counts.size:
            max_cnt = max(max_cnt, int(counts.max()))
    B_max = max(1, -(-max_cnt // P))
    C = NB * B_max

    wext = np.ascontiguousarray(np.concatenate([W, b[:, None]], axis=1).T)  # [65, 64]
    iota = np.ascontiguousarray(np.tile(np.arange(P, dtype=np.float32), (P, 1)))

    in_maps = []
    for c in range(NCORES):
        s_c, d_c, w_c, blk, counts = per_core[c]
        run_start = np.zeros(NB, dtype=np.int64)
        run_start[1:] = np.cumsum(counts)[:-1]
        within = np.arange(len(d_c), dtype=np.int64) - run_start[blk]
        pos = blk * (B_max * P) + within
        flat_src = np.zeros(C * P, dtype=np.int32)
        flat_rel = np.zeros(C * P, dtype=np.float32)
        flat_w = np.zeros(C * P, dtype=np.float32)
        flat_src[pos] = s_c
        flat_rel[pos] = (d_c & 127).astype(np.float32)
        flat_w[pos] = w_c
        in_maps.append(
            {
                "x": x,
                "srcT": np.ascontiguousarray(flat_src.reshape(C, P).T),
                "relT": np.ascontiguousarray(flat_rel.reshape(C, P).T),
                "wT": np.ascontiguousarray(flat_w.reshape(C, P).T),
                "wext": wext,
                "iota": iota,
            }
        )

    nc = _build_program(B_max)
    global _last_nc, _last_in_maps
    _last_nc, _last_in_maps = nc, in_maps
    results = run_bass_kernel_spmd(nc, in_maps, list(range(NCORES))).results
    out = np.concatenate(
        [results[c]["out"][:NODES_PER_CORE] for c in range(NCORES)], axis=0
    )
    return out.astype(np.float32)

